# revision 15
# baseline (speedup 1.0000x reference)
"""NGCF-style 2-layer GNN message passing on 8 Trainium2 NeuronCores.

Strategy (1D graph partition, per sharding hint):
  - Destination nodes (rows) split into 8 contiguous slabs of 6250; each
    core owns the edges whose destination falls in its slab.  A host-side
    LPT pass permutes node ids within each slab so 128-row windows have
    balanced edge counts (fixed k_lo/k_hi tiles of 128 edge slots per
    window; lo/hi = source-column < / >= 31250, the int16 gather-index
    split).
  - Layer-0 source features are gathered per edge in fp8-e4m3 (768 B
    rows) from a replicated quantized copy of ego; the destination-side
    ("+ego", Hadamard) path stays fp16, so only the summed neighbor
    messages see fp8 rounding (~1e-2 max rel err, harness gate 2e-2).
  - Gathers are merged: one SWDGE dma_gather per (group of 7 windows,
    lo/hi), with pad slots pointing at node 0 and val 0 -- counts are
    compile-time, no runtime count registers, and the 994 ns fixed SWDGE
    issue cost is paid ~28x instead of ~196x.
  - Segment-sum scatter runs "transposed": side^T[dchunk,row] is built
    directly by PE matmuls with the gathered tile as the stationary
    operand and a one-hot (val * [row==r]) DVE matrix as the moving
    operand.  This skips the separate transpose matmuls entirely; the
    linear layers then contract over feature chunks on the partition
    axis:  ego1 = lrelu(sideT+egoT @ W1 + sideT*egoT @ W2 + b).
  - The destination slab is consumed pre-transposed (host uploads
    egoT); layer-1's transposed slab (ego1T) is produced on device by
    one 128x128 PE transpose per window and persists in SBUF.
  - An on-device AllGather shares each core's ego1 slab (6250x128 fp16)
    for layer-1's global gather; layer 1 repeats the scheme at d=128
    (fp16 gather) -> 64.
  - Host assembles concat([ego, norm1, norm2], axis=1).
"""

import sys

sys.path.insert(0, "/opt/trn_rl_repo")

import numpy as np
import ml_dtypes
from contextlib import ExitStack

from concourse import bass, bacc, tile, masks
import concourse.mybir as mybir
from concourse.bass_utils import run_bass_kernel_spmd

f32 = mybir.dt.float32
f16 = mybir.dt.float16
fp8 = mybir.dt.float8e4
i16 = mybir.dt.int16

N_ITEMS = 30000
N_USERS = 20000
N = N_ITEMS + N_USERS          # 50000 nodes
E = 512000
D0, D1, D2 = 768, 128, 64
NCORE = 8
SLAB = N // NCORE              # 6250 rows per core
WIN = 128                      # rows per window
NW = (SLAB + WIN - 1) // WIN   # 49 windows per core
LAST_ROWS = SLAB - (NW - 1) * WIN  # 106 valid rows in last window
PAD_SLAB = NW * WIN            # 6272 (windows padded out)
HI = 31250                     # gather base split (max idx 18749 fits int16)
NPAIR = (NW + 1) // 2          # 25 window pairs (last is a single window)
EPS = 1e-12


def _nwp(p):
    """number of windows in pair p"""
    return min(2, NW - p * 2)


# ----------------------------------------------------------------------
# host-side edge preprocessing
# ----------------------------------------------------------------------

def _prep_edges(edge_rows, edge_cols, edge_vals):
    """LPT-balance rows into 128-row windows per core, permute node ids
    accordingly, and build padded device tables in gather-group order."""

    is_hi = (edge_cols >= HI).astype(np.int64)
    lo_cnt = np.bincount(edge_rows[is_hi == 0], minlength=N)
    hi_cnt = np.bincount(edge_rows[is_hi == 1], minlength=N)

    perm = np.empty(N, np.int64)        # new global pos -> orig node id
    for c in range(NCORE):
        lo_c = lo_cnt[c * SLAB : (c + 1) * SLAB]
        hi_c = hi_cnt[c * SLAB : (c + 1) * SLAB]
        tot_c = lo_c + hi_c
        order = np.argsort(-tot_c, kind="stable")
        caps = np.array([WIN] * (NW - 1) + [LAST_ROWS])
        wlo = np.zeros(NW)
        whi = np.zeros(NW)
        wn = np.zeros(NW, np.int64)
        tgt_lo = lo_c.sum() / NW
        tgt_hi = hi_c.sum() / NW
        wrows = [[] for _ in range(NW)]
        for r in order:
            feas = wn < caps
            rlo = (wlo + lo_c[r]) / max(tgt_lo, 1.0)
            rhi = (whi + hi_c[r]) / max(tgt_hi, 1.0)
            pen = np.maximum(rlo, rhi) * 1000.0 + (rlo + rhi)
            pen[~feas] = np.inf
            w = int(np.argmin(pen))
            wrows[w].append(r)
            wlo[w] += lo_c[r]
            whi[w] += hi_c[r]
            wn[w] += 1
        flat = np.concatenate([np.array(x, np.int64) for x in wrows])
        perm[c * SLAB : (c + 1) * SLAB] = c * SLAB + flat

    glob_pos = np.empty(N, np.int64)    # orig node id -> new global pos
    glob_pos[perm] = np.arange(N)

    prow = glob_pos[edge_rows]
    pcol = glob_pos[edge_cols]
    pcore = prow // SLAB
    plrow = prow - pcore * SLAB
    win = plrow // WIN
    rloc = (plrow - win * WIN).astype(np.float32)
    grp = (pcol >= HI).astype(np.int64)

    bucket = (pcore * NW + win) * 2 + grp
    order = np.argsort(bucket, kind="stable")
    b_sorted = bucket[order]
    counts = np.bincount(b_sorted, minlength=NCORE * NW * 2)
    starts = np.zeros_like(counts)
    np.cumsum(counts[:-1], out=starts[1:])
    rank = np.arange(E) - starts[b_sorted]

    cgrid = counts.reshape(NCORE, NW, 2)
    k_lo = max(1, int(np.ceil(cgrid[:, :, 0].max() / WIN)))
    k_hi = max(1, int(np.ceil(cgrid[:, :, 1].max() / WIN)))
    T = k_lo + k_hi
    ntiles = NW * T

    c_s = pcore[order]
    w_s = win[order]
    g_s = grp[order]
    col_s = (pcol[order] - HI * g_s).astype(np.int16)
    rloc_s = rloc[order]
    val_s = edge_vals[order]

    # tile column index in window-pair order (one dma_gather per <=1024
    # slots: lo per window [k_lo tiles], hi per pair [2*k_hi tiles]):
    #   pair: [lo(w0) k_lo][lo(w1) k_lo][hi(w0) k_hi][hi(w1) k_hi]
    pair = w_s // 2
    j_s = w_s % 2
    nwp_s = np.minimum(2, NW - pair * 2)
    pbase = pair * 2 * T                # first tile col of the pair
    tin = rank // WIN                   # tile index within (window, grp)
    tcol = np.where(
        g_s == 0,
        pbase + j_s * k_lo + tin,
        pbase + nwp_s * k_lo + j_s * k_hi + tin,
    )
    slot = rank % WIN

    idxbuf = np.zeros((NCORE, ntiles, WIN), np.int16)  # pad -> node 0 / HI
    rowsb = np.full((NCORE, ntiles, WIN), 255.0, np.float32)
    valsb = np.zeros((NCORE, ntiles, WIN), np.float32)
    idxbuf[c_s, tcol, slot] = col_s
    rowsb[c_s, tcol, slot] = rloc_s
    valsb[c_s, tcol, slot] = val_s

    # SWDGE index wrap: slot s of a tile -> [s % 16, tile*8 + s // 16]
    idx16 = idxbuf.reshape(NCORE, ntiles, 8, 16).transpose(0, 3, 1, 2)
    idx16 = idx16.reshape(NCORE, 16, ntiles * 8)
    idx16 = np.tile(idx16, (1, 8, 1))

    return {
        "k_lo": k_lo,
        "k_hi": k_hi,
        "idx16": np.ascontiguousarray(idx16),
        "rows": np.ascontiguousarray(rowsb.transpose(0, 2, 1)),
        "vals": np.ascontiguousarray(valsb.transpose(0, 2, 1)),
        "perm": perm,
    }


# ----------------------------------------------------------------------
# device program
# ----------------------------------------------------------------------

def _chunked_w(w):
    """(K, M) -> (128, (K//128)*M) stationary-chunk layout."""
    k, m = w.shape
    nk = k // 128
    return np.ascontiguousarray(
        w.reshape(nk, 128, m).transpose(1, 0, 2).reshape(128, nk * m)
    )


def _build_program(k_lo, k_hi, timing_variant=False):
    """timing_variant=True builds a single-core program (collective
    replaced by a local DMA) for TimelineSim cost analysis only."""
    T = k_lo + k_hi
    ntiles = NW * T
    nc = bacc.Bacc(
        "TRN2", target_bir_lowering=False, debug=False,
        num_devices=1 if timing_variant else NCORE,
    )

    ego8_d = nc.dram_tensor("ego8", [N, D0], fp8, kind="ExternalInput")
    egosT_d = nc.dram_tensor("egosT", [128, D0 // 128, PAD_SLAB], f16, kind="ExternalInput")
    w1_d = nc.dram_tensor("w1c", [128, 6 * D1], f16, kind="ExternalInput")
    w2_d = nc.dram_tensor("w2c", [128, 6 * D1], f16, kind="ExternalInput")
    b0_d = nc.dram_tensor("b0", [1, D1], f16, kind="ExternalInput")
    w11_d = nc.dram_tensor("w11", [D1, D2], f16, kind="ExternalInput")
    w21_d = nc.dram_tensor("w21", [D1, D2], f16, kind="ExternalInput")
    b1_d = nc.dram_tensor("b1", [1, D2], f16, kind="ExternalInput")
    idx_d = nc.dram_tensor("idxs", [128, ntiles * 8], i16, kind="ExternalInput")
    rows_d = nc.dram_tensor("rowsl", [128, ntiles], f32, kind="ExternalInput")
    vals_d = nc.dram_tensor("valsl", [128, ntiles], f32, kind="ExternalInput")
    iota_d = nc.dram_tensor("iota", [128, 128], f16, kind="ExternalInput")

    n1_d = nc.dram_tensor("n1", [PAD_SLAB, D1], f32, kind="ExternalOutput")
    n2_d = nc.dram_tensor("n2", [PAD_SLAB, D2], f32, kind="ExternalOutput")

    AL = mybir.AluOpType
    AF = mybir.ActivationFunctionType

    with tile.TileContext(nc) as tc, ExitStack() as ctx:
        const = ctx.enter_context(tc.tile_pool(name="const", bufs=1))
        dram = ctx.enter_context(tc.tile_pool(name="dram", bufs=1, space="DRAM"))

        ident = const.tile([128, 128], f16)
        masks.make_identity(nc, ident[:])
        ones1 = const.tile([1, 128], f16)
        nc.gpsimd.memset(ones1[:], 1.0)

        w1_t = const.tile([128, 6, D1], f16)
        w2_t = const.tile([128, 6, D1], f16)
        b0_t = const.tile([1, D1], f16)
        w11_t = const.tile([D1, D2], f16)
        w21_t = const.tile([D1, D2], f16)
        b1_t = const.tile([1, D2], f16)
        iota_t = const.tile([128, 128], f16)
        idx_t = const.tile([128, ntiles * 8], i16)
        rows_t = const.tile([128, ntiles], f32)
        vals_t = const.tile([128, ntiles], f32)
        ego1T = const.tile([128, NW, WIN], f16)  # persists L0 -> L1
        for sb, dr in [
            (w1_t.rearrange("p a b -> p (a b)"), w1_d),
            (w2_t.rearrange("p a b -> p (a b)"), w2_d),
            (b0_t[:], b0_d), (w11_t[:], w11_d), (w21_t[:], w21_d),
            (b1_t[:], b1_d), (iota_t[:], iota_d), (idx_t[:], idx_d),
            (rows_t[:], rows_d), (vals_t[:], vals_d),
        ]:
            nc.sync.dma_start(out=sb, in_=dr[:])

        ego1_slab16 = dram.tile([SLAB, D1], f16)
        ego1_full16 = dram.tile([N, D1], f16, addr_space="Shared")

        def layer(
            lctx, phase, src_full, gdt, d_in, nk, egoT_src, w1t, w2t, bt,
            d_out, out_norm_r, out_eg16,
        ):
            """One NGCF layer over all windows, grouped by G windows.

            src_full: DRAM AP (N, d_in) gather source (dtype gdt)
            egoT_src: 'dram' (load egosT_d per group) or SBUF AP provider
            out_norm_r: DRAM AP [128, NW, d_out] (rearranged) for norms
            out_eg16: None or (write ego1 slab16 + ego1T) for layer 0
            """
            gp = lctx.enter_context(tc.tile_pool(name=f"g{phase}", bufs=2))
            sp_ = lctx.enter_context(tc.tile_pool(name=f"s{phase}", bufs=2))
            wp = lctx.enter_context(tc.tile_pool(name=f"w{phase}", bufs=2))
            pseg = lctx.enter_context(
                tc.tile_pool(name=f"pseg{phase}", bufs=2, space="PSUM")
            )
            pout = lctx.enter_context(
                tc.tile_pool(name=f"pout{phase}", bufs=2, space="PSUM")
            )
            ptr = None
            if out_eg16 is not None:
                ptr = lctx.enter_context(
                    tc.tile_pool(name=f"ptr{phase}", bufs=2, space="PSUM")
                )
                r16 = ego1_slab16[0 : (NW - 1) * WIN, :].rearrange(
                    "(w p) m -> p w m", p=WIN
                )

            for p in range(NPAIR):
                pbase = p * 2 * T
                nwp = _nwp(p)
                glo0 = gp.tile([128, k_lo, d_in], gdt, tag="glo0")
                nc.gpsimd.dma_gather(
                    glo0[:], src_full,
                    idx_t[:, pbase * 8 : (pbase + k_lo) * 8],
                    k_lo * WIN, k_lo * WIN, d_in,
                )
                if nwp == 2:
                    glo1 = gp.tile([128, k_lo, d_in], gdt, tag="glo1")
                    nc.gpsimd.dma_gather(
                        glo1[:], src_full,
                        idx_t[:, (pbase + k_lo) * 8 : (pbase + 2 * k_lo) * 8],
                        k_lo * WIN, k_lo * WIN, d_in,
                    )
                hbase = pbase + nwp * k_lo
                ghi = gp.tile([128, nwp * k_hi, d_in], gdt, tag="ghi")
                nc.gpsimd.dma_gather(
                    ghi[:], src_full[HI:],
                    idx_t[:, hbase * 8 : (hbase + nwp * k_hi) * 8],
                    nwp * k_hi * WIN, nwp * k_hi * WIN, d_in,
                )

                if egoT_src == "dram":
                    egoT_g = sp_.tile([128, nk, nwp * WIN], f16, tag="egoT")
                    nc.sync.dma_start(
                        out=egoT_g[:],
                        in_=egosT_d[
                            :, :, p * 2 * WIN : p * 2 * WIN + nwp * WIN
                        ],
                    )

                if out_eg16 is not None:
                    eg16_g = sp_.tile([128, nwp, d_out], f16, tag="eg16g")
                no_g = sp_.tile([128, nwp, d_out], f32, tag="nog")

                for j in range(nwp):
                    w = p * 2 + j
                    onehot = wp.tile([128, T * 128], f16, tag="onehot")
                    cols_lo = [pbase + j * k_lo + t for t in range(k_lo)]
                    cols_hi = [
                        pbase + nwp * k_lo + j * k_hi + t for t in range(k_hi)
                    ]
                    cols = cols_lo + cols_hi
                    for ti, col in enumerate(cols):
                        nc.vector.tensor_scalar(
                            onehot[:, ti * 128 : (ti + 1) * 128],
                            iota_t[:],
                            rows_t[:, col : col + 1],
                            vals_t[:, col : col + 1],
                            AL.is_equal,
                            AL.mult,
                        )

                    psT = pseg.tile([128, nk, 128], f32, tag="psT")
                    glo = glo0 if j == 0 else glo1
                    # chunk-outer: one open PSUM accumulation group per
                    # bank at a time (interleaved groups within a bank
                    # accumulate incorrectly on HW)
                    for c in range(nk):
                        for ti in range(T):
                            if ti < k_lo:
                                gb, loc = glo, ti
                            else:
                                gb, loc = ghi, j * k_hi + (ti - k_lo)
                            oh = onehot[:, ti * 128 : (ti + 1) * 128]
                            nc.tensor.matmul(
                                psT[:, c, :],
                                gb[:, loc, c * 128 : (c + 1) * 128],
                                oh,
                                start=(ti == 0), stop=(ti == T - 1),
                            )

                    sideT = wp.tile([128, nk, 128], f16, tag="sideT")
                    nc.scalar.activation(sideT[:], psT[:], AF.Copy)
                    if egoT_src == "dram":
                        egoTw = egoT_g[:, :, j * WIN : (j + 1) * WIN]
                    else:
                        egoTw = ego1T[:, w : w + 1, :]
                    spT = wp.tile([128, nk, 128], f16, tag="spT")
                    hdT = wp.tile([128, nk, 128], f16, tag="hdT")
                    nc.vector.tensor_tensor(spT[:], sideT[:], egoTw, AL.add)
                    nc.vector.tensor_tensor(hdT[:], sideT[:], egoTw, AL.mult)

                    po = pout.tile([128, d_out], f32, tag="po")
                    for c in range(nk):
                        nc.tensor.matmul(
                            po[:], spT[:, c, :], w1t[:, c, :] if nk > 1 else w1t[:],
                            start=(c == 0), stop=False,
                        )
                    for c in range(nk):
                        nc.tensor.matmul(
                            po[:], hdT[:, c, :], w2t[:, c, :] if nk > 1 else w2t[:],
                            start=False, stop=False,
                        )
                    nc.tensor.matmul(
                        po[:], ones1[:], bt[:], start=False, stop=True
                    )

                    eg_t = wp.tile([128, d_out], f32, tag="eg")
                    # Prelu == leaky relu; it shares the 'sqrt_and_others'
                    # act-table set with Copy/Square/Sqrt (Lrelu does not),
                    # so the whole kernel needs one LoadActFuncSet.
                    nc.scalar.activation(eg_t[:], po[:], AF.Prelu, alpha=0.01)

                    if out_eg16 is not None:
                        nc.gpsimd.tensor_copy(eg16_g[:, j, :], eg_t[:])
                        ptt = ptr.tile([128, 128], f16, tag="ptt")
                        nc.tensor.matmul(
                            ptt[:], eg16_g[:, j, :], ident[:],
                            is_transpose=True, start=True, stop=True,
                        )
                        nc.vector.tensor_copy(ego1T[:, w, :], ptt[:])

                    # L2 normalize
                    sq_t = wp.tile([128, d_out], f32, tag="sq")
                    ss_t = wp.tile([128, 1], f32, tag="ss")
                    nc.scalar.activation(
                        sq_t[:], eg_t[:], AF.Square, accum_out=ss_t[:]
                    )
                    nrm_t = wp.tile([128, 1], f32, tag="nrm")
                    nc.scalar.activation(nrm_t[:], ss_t[:], AF.Sqrt)
                    nc.vector.tensor_scalar_max(nrm_t[:], nrm_t[:], EPS)
                    rcp_t = wp.tile([128, 1], f32, tag="rcp")
                    nc.vector.reciprocal(rcp_t[:], nrm_t[:])
                    nc.vector.tensor_scalar_mul(no_g[:, j, :], eg_t[:], rcp_t[:])

                # pair writes
                nc.sync.dma_start(
                    out=out_norm_r[:, p * 2 : p * 2 + nwp, :], in_=no_g[:]
                )
                if out_eg16 is not None:
                    if p < NPAIR - 1:
                        nc.sync.dma_start(
                            out=r16[:, p * 2 : p * 2 + nwp, :], in_=eg16_g[:]
                        )
                    else:
                        # last pair is the single partial window 48
                        nc.sync.dma_start(
                            out=ego1_slab16[(NW - 1) * WIN : SLAB, :],
                            in_=eg16_g[:LAST_ROWS, 0, :],
                        )

        n1_r = n1_d[:].rearrange("(w p) m -> p w m", p=WIN)
        n2_r = n2_d[:].rearrange("(w p) m -> p w m", p=WIN)

        # ---- layer 0 ----
        with ExitStack() as l0ctx:
            layer(
                l0ctx, "A", ego8_d[:], fp8, D0, 6, "dram", w1_t, w2_t, b0_t,
                D1, n1_r, True,
            )

        if timing_variant:
            nc.sync.dma_start(out=ego1_full16[0:SLAB, :], in_=ego1_slab16[:])
        else:
            nc.gpsimd.collective_compute(
                "AllGather",
                mybir.AluOpType.bypass,
                replica_groups=[list(range(NCORE))],
                ins=[ego1_slab16.opt()],
                outs=[ego1_full16.opt()],
            )

        # ---- layer 1 ----
        with ExitStack() as l1ctx:
            layer(
                l1ctx, "B", ego1_full16[:], f16, D1, 1, "sbuf", w11_t,
                w21_t, b1_t, D2, n2_r, None,
            )

    nc.compile()
    return nc


# ----------------------------------------------------------------------
# entry point
# ----------------------------------------------------------------------

def _prepare(
    item_embed, user_embed, W1_0, b1_0, W2_0, b2_0, W1_1, b1_1, W2_1, b2_1,
    edge_vals, edge_rows, edge_cols,
):
    item_embed = np.asarray(item_embed, np.float32)
    user_embed = np.asarray(user_embed, np.float32)
    edge_vals = np.asarray(edge_vals, np.float32)
    edge_rows = np.asarray(edge_rows, np.int32)
    edge_cols = np.asarray(edge_cols, np.int32)

    ego = np.concatenate([item_embed, user_embed], axis=0)
    prep = _prep_edges(edge_rows, edge_cols, edge_vals)
    k_lo, k_hi = prep["k_lo"], prep["k_hi"]
    perm = prep["perm"]
    ego_p = ego[perm]

    nc = _build_program(k_lo, k_hi)

    w1c = _chunked_w(np.asarray(W1_0, np.float32)).astype(np.float16)
    w2c = _chunked_w(np.asarray(W2_0, np.float32)).astype(np.float16)
    b0 = (np.asarray(b1_0, np.float32) + np.asarray(b2_0, np.float32))[None].astype(np.float16)
    w11 = np.ascontiguousarray(np.asarray(W1_1, np.float32)).astype(np.float16)
    w21 = np.ascontiguousarray(np.asarray(W2_1, np.float32)).astype(np.float16)
    b1 = (np.asarray(b1_1, np.float32) + np.asarray(b2_1, np.float32))[None].astype(np.float16)
    iota = np.ascontiguousarray(
        np.tile(np.arange(128, dtype=np.float16)[None], (128, 1))
    )

    ego8 = ego_p.astype(ml_dtypes.float8_e4m3)
    in_maps = []
    for c in range(NCORE):
        slab = ego_p[c * SLAB : (c + 1) * SLAB].astype(np.float16)
        slab_pad = np.zeros((PAD_SLAB, D0), np.float16)
        slab_pad[:SLAB] = slab
        egosT = np.ascontiguousarray(
            slab_pad.T.reshape(6, 128, PAD_SLAB).transpose(1, 0, 2)
        )
        in_maps.append({
            "ego8": ego8,
            "egosT": egosT,
            "w1c": w1c, "w2c": w2c, "b0": b0,
            "w11": w11, "w21": w21, "b1": b1,
            "idxs": prep["idx16"][c],
            "rowsl": prep["rows"][c],
            "valsl": prep["vals"][c],
            "iota": iota,
        })

    return nc, in_maps, ego, perm


LAST_EXEC_NS = None
LAST_TRACE = None


def kernel(**inputs):
    global LAST_EXEC_NS, LAST_TRACE
    nc, in_maps, ego, perm = _prepare(**inputs)
    res = run_bass_kernel_spmd(nc, in_maps, list(range(NCORE)))
    LAST_EXEC_NS = res.exec_time_ns
    if res.instructions_and_trace is not None:
        LAST_TRACE = res.instructions_and_trace[1]

    out = np.empty((N, D0 + D1 + D2), np.float32)
    out[:, :D0] = ego
    n1 = np.concatenate(
        [res.results[c]["n1"][:SLAB] for c in range(NCORE)], axis=0
    )
    n2 = np.concatenate(
        [res.results[c]["n2"][:SLAB] for c in range(NCORE)], axis=0
    )
    out[perm, D0 : D0 + D1] = n1
    out[perm, D0 + D1 :] = n2
    return out


# revision 24
# speedup vs baseline: 1.1360x; 1.1360x over previous
"""NGCF-style 2-layer GNN message passing on 8 Trainium2 NeuronCores.

Strategy (1D graph partition, per sharding hint):
  - Destination nodes (rows) split into 8 contiguous slabs of 6250; each
    core owns the edges whose destination falls in its slab.  A host-side
    LPT pass permutes node ids within each slab so 128-row windows have
    balanced edge counts (fixed k_lo/k_hi tiles of 128 edge slots per
    window; lo/hi = source-column < / >= 31250, the int16 gather-index
    split).
  - Layer-0 source features are gathered per edge in fp8-e4m3 (768 B
    rows) from a replicated quantized copy of ego; the destination-side
    ("+ego", Hadamard) path stays fp16, so only the summed neighbor
    messages see fp8 rounding (~1e-2 max rel err, harness gate 2e-2).
  - Gathers are merged: one SWDGE dma_gather per (group of 7 windows,
    lo/hi), with pad slots pointing at node 0 and val 0 -- counts are
    compile-time, no runtime count registers, and the 994 ns fixed SWDGE
    issue cost is paid ~28x instead of ~196x.
  - Segment-sum scatter runs "transposed": side^T[dchunk,row] is built
    directly by PE matmuls with the gathered tile as the stationary
    operand and a one-hot (val * [row==r]) DVE matrix as the moving
    operand.  This skips the separate transpose matmuls entirely; the
    linear layers then contract over feature chunks on the partition
    axis:  ego1 = lrelu(sideT+egoT @ W1 + sideT*egoT @ W2 + b).
  - The destination slab is consumed pre-transposed (host uploads
    egoT); layer-1's transposed slab (ego1T) is produced on device by
    one 128x128 PE transpose per window and persists in SBUF.
  - An on-device AllGather shares each core's ego1 slab (6250x128 fp16)
    for layer-1's global gather; layer 1 repeats the scheme at d=128
    (fp16 gather) -> 64.
  - Host assembles concat([ego, norm1, norm2], axis=1).
"""

import sys

sys.path.insert(0, "/opt/trn_rl_repo")

import numpy as np
import ml_dtypes
from contextlib import ExitStack

from concourse import bass, bacc, tile, masks
import concourse.mybir as mybir
from concourse.bass_utils import run_bass_kernel_spmd

f32 = mybir.dt.float32
f16 = mybir.dt.float16
fp8 = mybir.dt.float8e4
i16 = mybir.dt.int16

N_ITEMS = 30000
N_USERS = 20000
N = N_ITEMS + N_USERS          # 50000 nodes
E = 512000
D0, D1, D2 = 768, 128, 64
NCORE = 8
SLAB = N // NCORE              # 6250 rows per core
WIN = 128                      # rows per window
NW = (SLAB + WIN - 1) // WIN   # 49 windows per core
LAST_ROWS = SLAB - (NW - 1) * WIN  # 106 valid rows in last window
PAD_SLAB = NW * WIN            # 6272 (windows padded out)
HI = 31250                     # gather base split (max idx 18749 fits int16)
NPAIR = (NW + 1) // 2          # 25 window pairs (last is a single window)
EPS = 1e-12


def _nwp(p):
    """number of windows in pair p"""
    return min(2, NW - p * 2)


# ----------------------------------------------------------------------
# host-side edge preprocessing
# ----------------------------------------------------------------------

def _prep_edges(edge_rows, edge_cols, edge_vals):
    """LPT-balance rows into 128-row windows per core, permute node ids
    accordingly, and build padded device tables in gather-group order."""

    is_hi = (edge_cols >= HI).astype(np.int64)
    lo_cnt = np.bincount(edge_rows[is_hi == 0], minlength=N)
    hi_cnt = np.bincount(edge_rows[is_hi == 1], minlength=N)

    perm = np.empty(N, np.int64)        # new global pos -> orig node id
    for c in range(NCORE):
        lo_c = lo_cnt[c * SLAB : (c + 1) * SLAB]
        hi_c = hi_cnt[c * SLAB : (c + 1) * SLAB]
        tot_c = lo_c + hi_c
        order = np.argsort(-tot_c, kind="stable")
        caps = np.array([WIN] * (NW - 1) + [LAST_ROWS])
        wlo = np.zeros(NW)
        whi = np.zeros(NW)
        wn = np.zeros(NW, np.int64)
        tgt_lo = lo_c.sum() / NW
        tgt_hi = hi_c.sum() / NW
        wrows = [[] for _ in range(NW)]
        for r in order:
            feas = wn < caps
            rlo = (wlo + lo_c[r]) / max(tgt_lo, 1.0)
            rhi = (whi + hi_c[r]) / max(tgt_hi, 1.0)
            pen = np.maximum(rlo, rhi) * 1000.0 + (rlo + rhi)
            pen[~feas] = np.inf
            w = int(np.argmin(pen))
            wrows[w].append(r)
            wlo[w] += lo_c[r]
            whi[w] += hi_c[r]
            wn[w] += 1
        flat = np.concatenate([np.array(x, np.int64) for x in wrows])
        perm[c * SLAB : (c + 1) * SLAB] = c * SLAB + flat

    glob_pos = np.empty(N, np.int64)    # orig node id -> new global pos
    glob_pos[perm] = np.arange(N)

    prow = glob_pos[edge_rows]
    pcol = glob_pos[edge_cols]
    pcore = prow // SLAB
    plrow = prow - pcore * SLAB
    win = plrow // WIN
    rloc = (plrow - win * WIN).astype(np.float32)
    grp = (pcol >= HI).astype(np.int64)

    bucket = (pcore * NW + win) * 2 + grp
    order = np.argsort(bucket, kind="stable")
    b_sorted = bucket[order]
    counts = np.bincount(b_sorted, minlength=NCORE * NW * 2)
    starts = np.zeros_like(counts)
    np.cumsum(counts[:-1], out=starts[1:])
    rank = np.arange(E) - starts[b_sorted]

    cgrid = counts.reshape(NCORE, NW, 2)
    k_lo = max(1, int(np.ceil(cgrid[:, :, 0].max() / WIN)))
    k_hi = max(1, int(np.ceil(cgrid[:, :, 1].max() / WIN)))
    T = k_lo + k_hi
    ntiles = NW * T

    c_s = pcore[order]
    w_s = win[order]
    g_s = grp[order]
    col_s = (pcol[order] - HI * g_s).astype(np.int16)
    rloc_s = rloc[order]
    val_s = edge_vals[order]

    # tile column index in window-pair order (one dma_gather per <=1024
    # slots: lo per window [k_lo tiles], hi per pair [2*k_hi tiles]):
    #   pair: [lo(w0) k_lo][lo(w1) k_lo][hi(w0) k_hi][hi(w1) k_hi]
    pair = w_s // 2
    j_s = w_s % 2
    nwp_s = np.minimum(2, NW - pair * 2)
    pbase = pair * 2 * T                # first tile col of the pair
    tin = rank // WIN                   # tile index within (window, grp)
    tcol = np.where(
        g_s == 0,
        pbase + j_s * k_lo + tin,
        pbase + nwp_s * k_lo + j_s * k_hi + tin,
    )
    slot = rank % WIN

    idxbuf = np.zeros((NCORE, ntiles, WIN), np.int16)  # pad -> node 0 / HI
    rowsb = np.full((NCORE, ntiles, WIN), 255.0, np.float32)
    valsb = np.zeros((NCORE, ntiles, WIN), np.float32)
    idxbuf[c_s, tcol, slot] = col_s
    rowsb[c_s, tcol, slot] = rloc_s
    valsb[c_s, tcol, slot] = val_s

    # SWDGE index wrap: slot s of a tile -> [s % 16, tile*8 + s // 16]
    idx16 = idxbuf.reshape(NCORE, ntiles, 8, 16).transpose(0, 3, 1, 2)
    idx16 = idx16.reshape(NCORE, 16, ntiles * 8)
    idx16 = np.tile(idx16, (1, 8, 1))

    return {
        "k_lo": k_lo,
        "k_hi": k_hi,
        "idx16": np.ascontiguousarray(idx16),
        "rows": np.ascontiguousarray(rowsb.transpose(0, 2, 1)),
        "vals": np.ascontiguousarray(valsb.transpose(0, 2, 1)),
        "perm": perm,
    }


# ----------------------------------------------------------------------
# device program
# ----------------------------------------------------------------------

def _chunked_w(w):
    """(K, M) -> (128, (K//128)*M) stationary-chunk layout."""
    k, m = w.shape
    nk = k // 128
    return np.ascontiguousarray(
        w.reshape(nk, 128, m).transpose(1, 0, 2).reshape(128, nk * m)
    )


def _build_program(k_lo, k_hi, timing_variant=False):
    """timing_variant=True builds a single-core program (collective
    replaced by a local DMA) for TimelineSim cost analysis only."""
    T = k_lo + k_hi
    ntiles = NW * T
    nc = bacc.Bacc(
        "TRN2", target_bir_lowering=False, debug=False,
        num_devices=1 if timing_variant else NCORE,
    )

    ego8_d = nc.dram_tensor("ego8", [N, D0], fp8, kind="ExternalInput")
    egosT_d = nc.dram_tensor("egosT", [128, D0 // 128, PAD_SLAB], f16, kind="ExternalInput")
    w1_d = nc.dram_tensor("w1c", [128, 6 * D1], f16, kind="ExternalInput")
    w2_d = nc.dram_tensor("w2c", [128, 6 * D1], f16, kind="ExternalInput")
    b0_d = nc.dram_tensor("b0", [1, D1], f16, kind="ExternalInput")
    w11_d = nc.dram_tensor("w11", [D1, D2], f16, kind="ExternalInput")
    w21_d = nc.dram_tensor("w21", [D1, D2], f16, kind="ExternalInput")
    b1_d = nc.dram_tensor("b1", [1, D2], f16, kind="ExternalInput")
    idx_d = nc.dram_tensor("idxs", [128, ntiles * 8], i16, kind="ExternalInput")
    rows_d = nc.dram_tensor("rowsl", [128, ntiles], f32, kind="ExternalInput")
    vals_d = nc.dram_tensor("valsl", [128, ntiles], f32, kind="ExternalInput")
    iota_d = nc.dram_tensor("iota", [128, 128], f16, kind="ExternalInput")

    n1_d = nc.dram_tensor("n1", [PAD_SLAB, D1], f32, kind="ExternalOutput")
    n2_d = nc.dram_tensor("n2", [PAD_SLAB, D2], f32, kind="ExternalOutput")

    AL = mybir.AluOpType
    AF = mybir.ActivationFunctionType

    with tile.TileContext(nc) as tc, ExitStack() as ctx:
        const = ctx.enter_context(tc.tile_pool(name="const", bufs=1))
        dram = ctx.enter_context(tc.tile_pool(name="dram", bufs=1, space="DRAM"))

        ident = const.tile([128, 128], f16)
        masks.make_identity(nc, ident[:])
        ones1 = const.tile([1, 128], f16)
        nc.gpsimd.memset(ones1[:], 1.0)

        w1_t = const.tile([128, 6, D1], f16)
        w2_t = const.tile([128, 6, D1], f16)
        b0_t = const.tile([1, D1], f16)
        w11_t = const.tile([D1, D2], f16)
        w21_t = const.tile([D1, D2], f16)
        b1_t = const.tile([1, D2], f16)
        iota_t = const.tile([128, 128], f16)
        idx_t = const.tile([128, ntiles * 8], i16)
        rows_t = const.tile([128, ntiles], f32)
        vals_t = const.tile([128, ntiles], f32)
        ego1T = const.tile([128, NW, WIN], f16)  # persists L0 -> L1
        for sb, dr in [
            (w1_t.rearrange("p a b -> p (a b)"), w1_d),
            (w2_t.rearrange("p a b -> p (a b)"), w2_d),
            (b0_t[:], b0_d), (w11_t[:], w11_d), (w21_t[:], w21_d),
            (b1_t[:], b1_d), (iota_t[:], iota_d), (idx_t[:], idx_d),
            (rows_t[:], rows_d), (vals_t[:], vals_d),
        ]:
            nc.sync.dma_start(out=sb, in_=dr[:])

        ego1_slab16 = dram.tile([SLAB, D1], f16)
        ego1_full16 = dram.tile([N, D1], f16, addr_space="Shared")

        def layer(
            lctx, phase, src_full, gdt, d_in, nk, egoT_src, w1t, w2t, bt,
            d_out, out_norm_r, out_eg16,
        ):
            """One NGCF layer over all windows, grouped by G windows.

            src_full: DRAM AP (N, d_in) gather source (dtype gdt)
            egoT_src: 'dram' (load egosT_d per group) or SBUF AP provider
            out_norm_r: DRAM AP [128, NW, d_out] (rearranged) for norms
            out_eg16: None or (write ego1 slab16 + ego1T) for layer 0
            """
            gp = lctx.enter_context(tc.tile_pool(name=f"g{phase}", bufs=2))
            sp_ = lctx.enter_context(tc.tile_pool(name=f"s{phase}", bufs=2))
            wp = lctx.enter_context(tc.tile_pool(name=f"w{phase}", bufs=2))
            pseg = lctx.enter_context(
                tc.tile_pool(name=f"pseg{phase}", bufs=2, space="PSUM")
            )
            pout = lctx.enter_context(
                tc.tile_pool(name=f"pout{phase}", bufs=2, space="PSUM")
            )
            ptr = None
            if out_eg16 is not None:
                ptr = lctx.enter_context(
                    tc.tile_pool(name=f"ptr{phase}", bufs=2, space="PSUM")
                )
                r16 = ego1_slab16[0 : (NW - 1) * WIN, :].rearrange(
                    "(w p) m -> p w m", p=WIN
                )

            # software pipeline state: pending = finish-work of window w-1,
            # tq = eg16 slice of window w-2 awaiting its PE transpose,
            # oh_next = pre-built onehot for the next window
            st = {"pending": None, "tq": None, "oh_next": None}

            def build_onehot(w):
                p_, j_ = w // 2, w % 2
                nwp_ = _nwp(p_)
                pb = p_ * 2 * T
                oht = wp.tile([128, T * 128], f16, tag="onehot")
                cols = [pb + j_ * k_lo + t for t in range(k_lo)] + [
                    pb + nwp_ * k_lo + j_ * k_hi + t for t in range(k_hi)
                ]
                for ti, col in enumerate(cols):
                    nc.vector.tensor_scalar(
                        oht[:, ti * 128 : (ti + 1) * 128],
                        iota_t[:],
                        rows_t[:, col : col + 1],
                        vals_t[:, col : col + 1],
                        AL.is_equal,
                        AL.mult,
                    )
                return oht

            def emit_tq_transpose():
                tw, tslice = st["tq"]
                ptt = ptr.tile([128, 128], f16, tag="ptt")
                nc.tensor.matmul(
                    ptt[:], tslice, ident[:],
                    is_transpose=True, start=True, stop=True,
                )
                nc.vector.tensor_copy(ego1T[:, tw, :], ptt[:])
                st["tq"] = None

            def finish(fc):
                w, j = fc["w"], fc["j"]
                spT, hdT = fc["spT"], fc["hdT"]
                po = pout.tile([128, d_out], f32, tag="po")
                for c in range(nk):
                    nc.tensor.matmul(
                        po[:], spT[:, c, :],
                        w1t[:, c, :] if nk > 1 else w1t[:],
                        start=(c == 0), stop=False,
                    )
                for c in range(nk):
                    nc.tensor.matmul(
                        po[:], hdT[:, c, :],
                        w2t[:, c, :] if nk > 1 else w2t[:],
                        start=False, stop=False,
                    )
                nc.tensor.matmul(
                    po[:], ones1[:], bt[:], start=False, stop=True
                )
                # transpose of window w-1's ego1 (its eg16 copy has long
                # landed) rides in this PE batch
                if out_eg16 is not None and st["tq"] is not None:
                    emit_tq_transpose()

                eg_t = wp.tile([128, d_out], f32, tag="eg")
                # Prelu == leaky relu; it shares the 'sqrt_and_others'
                # act-table set with Copy/Square/Sqrt (Lrelu does not), so
                # the whole kernel needs one LoadActFuncSet.
                nc.scalar.activation(eg_t[:], po[:], AF.Prelu, alpha=0.01)

                if out_eg16 is not None:
                    nc.gpsimd.tensor_copy(fc["eg16_g"][:, j, :], eg_t[:])
                    st["tq"] = (w, fc["eg16_g"][:, j, :])

                # L2 normalize
                sq_t = wp.tile([128, d_out], f32, tag="sq")
                ss_t = wp.tile([128, 1], f32, tag="ss")
                nc.scalar.activation(
                    sq_t[:], eg_t[:], AF.Square, accum_out=ss_t[:]
                )
                nrm_t = wp.tile([128, 1], f32, tag="nrm")
                nc.scalar.activation(nrm_t[:], ss_t[:], AF.Sqrt)
                nc.vector.tensor_scalar_max(nrm_t[:], nrm_t[:], EPS)
                rcp_t = wp.tile([128, 1], f32, tag="rcp")
                nc.vector.reciprocal(rcp_t[:], nrm_t[:])
                nc.vector.tensor_scalar_mul(
                    fc["no_g"][:, j, :], eg_t[:], rcp_t[:]
                )

                if fc["pair_write"] is not None:
                    pw, nwp_ = fc["pair_write"]
                    nc.sync.dma_start(
                        out=out_norm_r[:, pw * 2 : pw * 2 + nwp_, :],
                        in_=fc["no_g"][:],
                    )
                    if out_eg16 is not None:
                        if pw < NPAIR - 1:
                            nc.sync.dma_start(
                                out=r16[:, pw * 2 : pw * 2 + nwp_, :],
                                in_=fc["eg16_g"][:],
                            )
                        else:
                            # last pair is the single partial window 48
                            nc.sync.dma_start(
                                out=ego1_slab16[(NW - 1) * WIN : SLAB, :],
                                in_=fc["eg16_g"][:LAST_ROWS, 0, :],
                            )

            for p in range(NPAIR):
                pbase = p * 2 * T
                nwp = _nwp(p)
                glo0 = gp.tile([128, k_lo, d_in], gdt, tag="glo0")
                nc.gpsimd.dma_gather(
                    glo0[:], src_full,
                    idx_t[:, pbase * 8 : (pbase + k_lo) * 8],
                    k_lo * WIN, k_lo * WIN, d_in,
                )
                if nwp == 2:
                    glo1 = gp.tile([128, k_lo, d_in], gdt, tag="glo1")
                    nc.gpsimd.dma_gather(
                        glo1[:], src_full,
                        idx_t[:, (pbase + k_lo) * 8 : (pbase + 2 * k_lo) * 8],
                        k_lo * WIN, k_lo * WIN, d_in,
                    )
                hbase = pbase + nwp * k_lo
                ghi = gp.tile([128, nwp * k_hi, d_in], gdt, tag="ghi")
                nc.gpsimd.dma_gather(
                    ghi[:], src_full[HI:],
                    idx_t[:, hbase * 8 : (hbase + nwp * k_hi) * 8],
                    nwp * k_hi * WIN, nwp * k_hi * WIN, d_in,
                )

                if egoT_src == "dram":
                    egoT_g = sp_.tile([128, nk, nwp * WIN], f16, tag="egoT")
                    nc.sync.dma_start(
                        out=egoT_g[:],
                        in_=egosT_d[
                            :, :, p * 2 * WIN : p * 2 * WIN + nwp * WIN
                        ],
                    )

                if out_eg16 is not None:
                    eg16_g = sp_.tile([128, nwp, d_out], f16, tag="eg16g")
                no_g = sp_.tile([128, nwp, d_out], f32, tag="nog")

                for j in range(nwp):
                    w = p * 2 + j
                    if st["oh_next"] is not None:
                        onehot = st["oh_next"]
                    else:
                        onehot = build_onehot(w)

                    psT = pseg.tile([128, nk, 128], f32, tag="psT")
                    glo = glo0 if j == 0 else glo1
                    # chunk-outer: one open PSUM accumulation group per
                    # bank at a time (interleaved groups within a bank
                    # accumulate incorrectly on HW)
                    for c in range(nk):
                        for ti in range(T):
                            if ti < k_lo:
                                gb, loc = glo, ti
                            else:
                                gb, loc = ghi, j * k_hi + (ti - k_lo)
                            oh = onehot[:, ti * 128 : (ti + 1) * 128]
                            nc.tensor.matmul(
                                psT[:, c, :],
                                gb[:, loc, c * 128 : (c + 1) * 128],
                                oh,
                                start=(ti == 0), stop=(ti == T - 1),
                            )

                    # onehot for the NEXT window builds on DVE while PE
                    # streams this window's scatter (it only reads const
                    # tables, so it is never blocked)
                    st["oh_next"] = build_onehot(w + 1) if w + 1 < NW else None

                    sideT = wp.tile([128, nk, 128], f16, tag="sideT")
                    nc.scalar.activation(sideT[:], psT[:], AF.Copy)
                    if egoT_src == "dram":
                        egoTw = egoT_g[:, :, j * WIN : (j + 1) * WIN]
                    else:
                        egoTw = ego1T[:, w : w + 1, :]
                    spT = wp.tile([128, nk, 128], f16, tag="spT")
                    hdT = wp.tile([128, nk, 128], f16, tag="hdT")
                    nc.vector.tensor_tensor(spT[:], sideT[:], egoTw, AL.add)
                    nc.vector.tensor_tensor(hdT[:], sideT[:], egoTw, AL.mult)

                    fc = {
                        "w": w, "spT": spT, "hdT": hdT,
                        "no_g": no_g, "j": j,
                        "eg16_g": eg16_g if out_eg16 is not None else None,
                        "pair_write": (p, nwp) if j == nwp - 1 else None,
                    }
                    # finish the PREVIOUS window after this scatter so PE
                    # never waits on the Act/DVE round-trip
                    if st["pending"] is not None:
                        finish(st["pending"])
                    st["pending"] = fc

            # drain the pipeline
            finish(st["pending"])
            st["pending"] = None
            if out_eg16 is not None and st["tq"] is not None:
                emit_tq_transpose()

        n1_r = n1_d[:].rearrange("(w p) m -> p w m", p=WIN)
        n2_r = n2_d[:].rearrange("(w p) m -> p w m", p=WIN)

        # ---- layer 0 ----
        with ExitStack() as l0ctx:
            layer(
                l0ctx, "A", ego8_d[:], fp8, D0, 6, "dram", w1_t, w2_t, b0_t,
                D1, n1_r, True,
            )

        if timing_variant:
            nc.sync.dma_start(out=ego1_full16[0:SLAB, :], in_=ego1_slab16[:])
        else:
            nc.gpsimd.collective_compute(
                "AllGather",
                mybir.AluOpType.bypass,
                replica_groups=[list(range(NCORE))],
                ins=[ego1_slab16.opt()],
                outs=[ego1_full16.opt()],
            )

        # ---- layer 1 ----
        with ExitStack() as l1ctx:
            layer(
                l1ctx, "B", ego1_full16[:], f16, D1, 1, "sbuf", w11_t,
                w21_t, b1_t, D2, n2_r, None,
            )

    nc.compile()
    return nc


# ----------------------------------------------------------------------
# entry point
# ----------------------------------------------------------------------

def _prepare(
    item_embed, user_embed, W1_0, b1_0, W2_0, b2_0, W1_1, b1_1, W2_1, b2_1,
    edge_vals, edge_rows, edge_cols,
):
    item_embed = np.asarray(item_embed, np.float32)
    user_embed = np.asarray(user_embed, np.float32)
    edge_vals = np.asarray(edge_vals, np.float32)
    edge_rows = np.asarray(edge_rows, np.int32)
    edge_cols = np.asarray(edge_cols, np.int32)

    ego = np.concatenate([item_embed, user_embed], axis=0)
    prep = _prep_edges(edge_rows, edge_cols, edge_vals)
    k_lo, k_hi = prep["k_lo"], prep["k_hi"]
    perm = prep["perm"]
    ego_p = ego[perm]

    nc = _build_program(k_lo, k_hi)

    w1c = _chunked_w(np.asarray(W1_0, np.float32)).astype(np.float16)
    w2c = _chunked_w(np.asarray(W2_0, np.float32)).astype(np.float16)
    b0 = (np.asarray(b1_0, np.float32) + np.asarray(b2_0, np.float32))[None].astype(np.float16)
    w11 = np.ascontiguousarray(np.asarray(W1_1, np.float32)).astype(np.float16)
    w21 = np.ascontiguousarray(np.asarray(W2_1, np.float32)).astype(np.float16)
    b1 = (np.asarray(b1_1, np.float32) + np.asarray(b2_1, np.float32))[None].astype(np.float16)
    iota = np.ascontiguousarray(
        np.tile(np.arange(128, dtype=np.float16)[None], (128, 1))
    )

    ego8 = ego_p.astype(ml_dtypes.float8_e4m3)
    in_maps = []
    for c in range(NCORE):
        slab = ego_p[c * SLAB : (c + 1) * SLAB].astype(np.float16)
        slab_pad = np.zeros((PAD_SLAB, D0), np.float16)
        slab_pad[:SLAB] = slab
        egosT = np.ascontiguousarray(
            slab_pad.T.reshape(6, 128, PAD_SLAB).transpose(1, 0, 2)
        )
        in_maps.append({
            "ego8": ego8,
            "egosT": egosT,
            "w1c": w1c, "w2c": w2c, "b0": b0,
            "w11": w11, "w21": w21, "b1": b1,
            "idxs": prep["idx16"][c],
            "rowsl": prep["rows"][c],
            "valsl": prep["vals"][c],
            "iota": iota,
        })

    return nc, in_maps, ego, perm


LAST_EXEC_NS = None
LAST_TRACE = None


def kernel(**inputs):
    global LAST_EXEC_NS, LAST_TRACE
    nc, in_maps, ego, perm = _prepare(**inputs)
    res = run_bass_kernel_spmd(nc, in_maps, list(range(NCORE)))
    LAST_EXEC_NS = res.exec_time_ns
    if res.instructions_and_trace is not None:
        LAST_TRACE = res.instructions_and_trace[1]

    out = np.empty((N, D0 + D1 + D2), np.float32)
    out[:, :D0] = ego
    n1 = np.concatenate(
        [res.results[c]["n1"][:SLAB] for c in range(NCORE)], axis=0
    )
    n2 = np.concatenate(
        [res.results[c]["n2"][:SLAB] for c in range(NCORE)], axis=0
    )
    out[perm, D0 : D0 + D1] = n1
    out[perm, D0 + D1 :] = n2
    return out
